# revision 25
# baseline (speedup 1.0000x reference)
"""MoE layer (16 experts, top-2) on 8 Trainium2 NeuronCores, FF-shard parallel.

Strategy:
  - Host computes gating (logits -> top-k -> softmax) and buckets tokens by
    expert (dispatch).
  - Each expert's FFN is split into NSH column shards (D_FF/NSH columns of
    W1 / rows of W2).  All 16*NSH shard-items are sorted by token count and
    dealt into NSLOT = 16*NSH/8 slots of 8 cores each; a slot's compile-time
    width is the max count in its group.  This nearly removes the padding
    imbalance of whole-expert slots (sum of slot widths ~1037 token-pairs
    per core for NSH=4 vs 1078, ideal 1024) at no extra weight traffic:
    shard weights shrink by NSH.
  - DMA is descriptor-rate bound (~25ns + bytes/24GBps per descriptor per
    engine), so every tensor is laid out partition-major with the whole
    slot's data contiguous per partition: x, w1, w2, y all move in 1-2
    DMAs of 8-16KB-per-partition runs (128 descriptors each) per slot.
  - Per slot: ht = silu(W1sh.T @ xt + b1sh) (f on partitions), then partial
    y = W2sh.T @ ht (d on partitions).  Host sums the NSH partials per
    expert and applies the gate weights (combine).
  - All matmuls bf16 (full PE rate, FWL), fp32 PSUM.  y stored bf16
    (partials are small; rounding adds ~0.1% rms, inside the budget).
  - Startup: first W1 quarter rides scalar in parallel with x kd-chunks on
    sync; first mm1 group sub-tiled so the PE starts on the first ~40KB.
    Drain: last slot's final kd group sub-tiled, stores rotate engines.
"""

import math
import os

import numpy as np

D_MODEL = 1024
D_FF = 4096
N_EXPERTS = 16
N_CORES = 8
KD = D_MODEL // 128  # 8 contraction chunks for mm1 / output chunks for mm2

NSH = int(os.environ.get("MOE_NSH", "8"))  # FF shards per expert

_PROG_CACHE: dict[tuple, object] = {}


def _split_tokens(c, cap=512):
    """Split token count c into moving-dim tiles <= cap (PSUM bank limit),
    as equal as possible."""
    n = max(1, math.ceil(c / cap))
    q, r = divmod(c, n)
    sizes = [q + (1 if i < r else 0) for i in range(n)]
    out = []
    c0 = 0
    for sz in sizes:
        out.append((c0, sz))
        c0 += sz
    return out


def _build_program(CS, nsh):
    import concourse.bass as bass  # noqa: F401
    import concourse.tile as tile
    from concourse import bacc, mybir

    f32 = mybir.dt.float32
    bf16 = mybir.dt.bfloat16
    silu = mybir.ActivationFunctionType.Silu

    KFS = (D_FF // nsh) // 128  # f chunks per shard
    NSLOT = len(CS)

    nc = bacc.Bacc("TRN2", target_bir_lowering=False, debug=False, num_devices=N_CORES)

    xt_d, y_d, w1_d, w2_d = [], [], [], []
    for j, C in enumerate(CS):
        xt_d.append(nc.dram_tensor(f"xt{j}", [128, KD, C], bf16, kind="ExternalInput").ap())
        y_d.append(nc.dram_tensor(f"y{j}", [128, KD, C], bf16, kind="ExternalOutput").ap())
        w1_d.append(nc.dram_tensor(f"w1_{j}", [128, KFS, KD, 128], bf16, kind="ExternalInput").ap())
        w2_d.append(nc.dram_tensor(f"w2_{j}", [128, KD, KFS, 128], bf16, kind="ExternalInput").ap())
    b1_d = nc.dram_tensor("b1", [128, NSLOT * KFS], f32, kind="ExternalInput").ap()

    with tile.TileContext(nc) as tc:
        with (
            tc.tile_pool(name="xtp", bufs=3) as xtp,
            tc.tile_pool(name="w1p", bufs=2) as w1p,
            tc.tile_pool(name="w2p", bufs=2) as w2p,
            tc.tile_pool(name="htp", bufs=1) as htp,
            tc.tile_pool(name="ytp", bufs=2) as ytp,
            tc.tile_pool(name="smallp", bufs=1) as smallp,
            tc.tile_pool(name="ps1", bufs=4, space="PSUM") as ps1,
            tc.tile_pool(name="ps2", bufs=4, space="PSUM") as ps2,
        ):
            b1t = smallp.tile([128, NSLOT * KFS], f32, name="b1t", tag="b1t")
            xts = {}

            def load_x(j, first=False):
                C = CS[j]
                xts[j] = xtp.tile([128, KD, C], bf16, name=f"xt{j}", tag="xt")
                if first:
                    # mm1 of slot 0 runs in two token-half passes, so only
                    # the left half of x is needed up front.  kd-pair chunks
                    # in consumption order (kd descending) spread over the
                    # rings; the first w1 chunk rides scalar in parallel.
                    h = (C // 2 + 1) // 2 * 2
                    for lo, hi, c0, c1, eng in (
                        (6, 8, 0, h, nc.sync), (6, 8, h, C, nc.sync),
                        (4, 6, 0, h, nc.scalar), (0, 2, 0, h, nc.scalar),
                        (2, 4, 0, h, nc.gpsimd), (4, 6, h, C, nc.gpsimd),
                        (2, 4, h, C, nc.gpsimd), (0, 2, h, C, nc.scalar),
                    ):
                        eng.dma_start(xts[j][:, lo:hi, c0:c1], xt_d[j][:, lo:hi, c0:c1])
                else:
                    eng = nc.gpsimd if j % 2 == 0 else nc.scalar
                    eng.dma_start(xts[j][:], xt_d[j][:])

            for j, C in enumerate(CS):
                tiles = _split_tokens(C)
                last_slot = j == NSLOT - 1

                # ---- loads ----
                w1t = w1p.tile([128, KFS, KD, 128], bf16, name=f"w1t{j}", tag="w1t")
                w2t = w2p.tile([128, KD, KFS, 128], bf16, name=f"w2t{j}", tag="w2t")
                if j == 0:
                    # startup: first w1 chunk on scalar in parallel with x
                    # chunks; remaining w1 chunks per-kf on sync at the
                    # kf-consumption cadence; w2 halves ride scalar+gpsimd
                    # behind the x chunks so they fill in parallel with the
                    # sync w1 stream and are in place for mm2(0).
                    nc.scalar.dma_start(w1t[:, 0:1], w1_d[0][:, 0:1])
                    nc.gpsimd.dma_start(b1t[:], b1_d)
                    load_x(0, first=True)
                    for kf in range(1, KFS):
                        nc.sync.dma_start(w1t[:, kf : kf + 1], w1_d[0][:, kf : kf + 1])
                    nc.scalar.dma_start(w2t[:, 0:4], w2_d[0][:, 0:4])
                    nc.gpsimd.dma_start(w2t[:, 4:KD], w2_d[0][:, 4:KD])
                else:
                    h = KFS // 2
                    nc.sync.dma_start(w1t[:, 0:h], w1_d[j][:, 0:h])
                    nc.sync.dma_start(w1t[:, h:KFS], w1_d[j][:, h:KFS])
                    # w2 halves on scalar+gpsimd (off sync, which carries w1)
                    nc.scalar.dma_start(w2t[:, 0:4], w2_d[j][:, 0:4])
                    nc.gpsimd.dma_start(w2t[:, 4:KD], w2_d[j][:, 4:KD])

                # ---- mm1: ht[f, c] = silu(W1sh.T @ xt + b1sh) ----
                xt = xts[j]
                ht = htp.tile([128, KFS, C], bf16, name=f"ht{j}", tag="ht")
                # slot 0 runs mm1 one token-tile per pass (so only part of x
                # is needed up front); later slots do all tiles per kf group
                passes = [[t] for t in tiles] if j == 0 else [tiles]
                for pi, ptiles in enumerate(passes):
                    for kf in range(KFS):
                        # sub-tile the very first group so the PE starts as
                        # soon as the first slice of x lands
                        if j == 0 and pi == 0 and kf == 0:
                            c0_, tw_ = ptiles[0]
                            mtiles = [(c0_ + o, w) for o, w in _split_tokens(tw_, 144)]
                        else:
                            mtiles = ptiles
                        pt = [
                            ps1.tile([128, 512], f32, name=f"ps1_{j}_{pi}_{kf}_{i}", tag="ps1")
                            for i in range(len(mtiles))
                        ]
                        for jj, kd in enumerate(reversed(range(KD))):
                            for p, (c0, tw) in zip(pt, mtiles):
                                nc.tensor.matmul(
                                    p[:, :tw],
                                    lhsT=w1t[:, kf, kd],
                                    rhs=xt[:, kd, c0 : c0 + tw],
                                    start=(jj == 0),
                                    stop=(jj == KD - 1),
                                )
                        for p, (c0, tw) in zip(pt, mtiles):
                            nc.scalar.activation(
                                ht[:, kf, c0 : c0 + tw],
                                p[:, :tw],
                                silu,
                                bias=b1t[:, j * KFS + kf : j * KFS + kf + 1],
                            )

                # next slot's x: issued here so it queues behind this slot's
                # w2 halves on its ring and lands during this slot's mm2
                if j + 1 <= NSLOT - 1:
                    load_x(j + 1)

                # ---- mm2: y[d, c] = W2sh.T @ ht (partial over this shard) ----
                yt = ytp.tile([128, KD, C], bf16, name=f"yt{j}", tag="yt")
                for kd in range(KD):
                    fine = last_slot and kd == KD - 1
                    mtiles = _split_tokens(C, 128) if fine else tiles
                    pt2 = [
                        ps2.tile([128, 512], f32, name=f"ps2_{j}_{kd}_{i}", tag="ps2")
                        for i in range(len(mtiles))
                    ]
                    for kf in range(KFS):
                        for p, (c0, tw) in zip(pt2, mtiles):
                            nc.tensor.matmul(
                                p[:, :tw],
                                lhsT=w2t[:, kd, kf],
                                rhs=ht[:, kf, c0 : c0 + tw],
                                start=(kf == 0),
                                stop=(kf == KFS - 1),
                            )
                    for i, (p, (c0, tw)) in enumerate(zip(pt2, mtiles)):
                        if fine:
                            # all copies on vector (fast CAST, free queue, and
                            # keeps them off the queues doing DIRECT2D issue);
                            # stores on sync/scalar only: gpsimd's
                            # end-of-program ring drain is slow
                            nc.vector.tensor_copy(yt[:, kd, c0 : c0 + tw], p[:, :tw])
                            deng = nc.sync if i % 2 == 0 else nc.scalar
                            deng.dma_start(y_d[j][:, kd, c0 : c0 + tw], yt[:, kd, c0 : c0 + tw])
                        else:
                            nc.vector.tensor_copy(yt[:, kd, c0 : c0 + tw], p[:, :tw])
                    # whole-half stores: one DMA per 4 kd chunks (contiguous
                    # per partition), alternating rings per slot.  The last
                    # slot's stores stay OFF gpsimd (its end-of-program ring
                    # drain is ~5us after its last DMA) — sync is free there
                    # since the w1 stream is done.
                    if kd == 3:
                        e1 = nc.sync if last_slot else (nc.scalar if j % 2 == 0 else nc.gpsimd)
                        e1.dma_start(y_d[j][:, 0:4], yt[:, 0:4])
                    elif kd == KD - 1 and not last_slot:
                        e2 = nc.gpsimd if j % 2 == 0 else nc.scalar
                        e2.dma_start(y_d[j][:, 4:KD], yt[:, 4:KD])
                    elif last_slot and kd == KD - 2:
                        nc.scalar.dma_start(y_d[j][:, 4:KD - 1], yt[:, 4:KD - 1])

    nc.compile()
    return nc


def _route(x2d, Wg, k):
    logits = x2d.astype(np.float32) @ Wg.astype(np.float32)  # [T, E]
    idx = np.argsort(-logits, axis=1, kind="stable")[:, :k]  # [T, k]
    vals = np.take_along_axis(logits, idx, axis=1)
    e = np.exp(vals - vals.max(axis=1, keepdims=True))
    w = (e / e.sum(axis=1, keepdims=True)).astype(np.float32)
    return idx, w


def kernel(x, W1, b1, W2, b2, Wg, k):
    import ml_dtypes
    from concourse.bass_utils import run_bass_kernel_spmd

    bf16 = ml_dtypes.bfloat16

    x = np.asarray(x, np.float32)
    W1 = np.asarray(W1, np.float32)
    b1 = np.asarray(b1, np.float32)
    W2 = np.asarray(W2, np.float32)
    b2 = np.asarray(b2, np.float32)
    Wg = np.asarray(Wg, np.float32)
    k = int(k)

    B, T, D = x.shape
    x2d = np.ascontiguousarray(x.reshape(-1, D))
    n_tok = x2d.shape[0]

    idx, w = _route(x2d, Wg, k)

    # bucket tokens per expert
    tok_lists, wt_lists = [], []
    for e in range(N_EXPERTS):
        sel = np.nonzero(idx == e)
        tok_lists.append(sel[0].astype(np.int64))
        wt_lists.append(w[sel[0], sel[1]])

    counts = np.array([len(t) for t in tok_lists])

    # shard items: NSH FF-shards per expert, sorted by count desc, dealt into
    # NSLOT groups of 8 (one item per core per slot)
    items = [(e, q) for e in range(N_EXPERTS) for q in range(NSH)]
    items.sort(key=lambda t: (-counts[t[0]], t[0], t[1]))
    NSLOT = len(items) // N_CORES

    def _pad(c):
        return max(16, ((int(c) + 1) // 2) * 2)

    slot_items, CS = [], []
    for j in range(NSLOT):
        grp = items[N_CORES * j : N_CORES * (j + 1)]
        slot_items.append(grp)
        CS.append(_pad(max(counts[e] for e, _ in grp)))

    key = (NSH, tuple(CS))
    nc = _PROG_CACHE.get(key)
    if nc is None:
        nc = _build_program(CS, NSH)
        _PROG_CACHE[key] = nc

    # host-side weight relayout (bf16, partition-major contiguous blocks)
    KFS = (D_FF // NSH) // 128
    # w1_host[e, q]: [128, KFS, KD, 128];  [p, kf, kd, p2] = W1[e, kd*128+p, q*FSH+kf*128+p2]
    w1_host = np.ascontiguousarray(
        W1.reshape(N_EXPERTS, KD, 128, NSH, KFS, 128).transpose(0, 3, 2, 4, 1, 5)
    ).astype(bf16)
    # w2_host[e, q]: [128, KD, KFS, 128];  [p, kd, kf, p2] = W2[e, q*FSH+kf*128+p, kd*128+p2]
    w2_host = np.ascontiguousarray(
        W2.reshape(N_EXPERTS, NSH, KFS, 128, KD, 128).transpose(0, 1, 3, 4, 2, 5)
    ).astype(bf16)
    # b1_host[e, q]: [128, KFS]
    b1_host = np.ascontiguousarray(
        b1.reshape(N_EXPERTS, NSH, KFS, 128).transpose(0, 1, 3, 2)
    )
    x_bf = x2d.astype(bf16)

    in_maps = []
    for c in range(N_CORES):
        b1all = np.zeros((128, NSLOT * KFS), np.float32)
        m = {"b1": b1all}
        for j in range(NSLOT):
            e, q = slot_items[j][c]
            toks = tok_lists[e]
            cnt = len(toks)
            C = CS[j]
            xt = np.zeros((128, KD, C), bf16)
            # xt[p, kd, c] = x[token c, kd*128 + p]
            xt[:, :, :cnt] = x_bf[toks].reshape(cnt, KD, 128).transpose(2, 1, 0)
            m[f"xt{j}"] = xt
            m[f"w1_{j}"] = w1_host[e, q]
            m[f"w2_{j}"] = w2_host[e, q]
            b1all[:, j * KFS : (j + 1) * KFS] = b1_host[e, q]
        in_maps.append(m)

    trace = bool(os.environ.get("MOE_TRACE"))
    r = run_bass_kernel_spmd(nc, in_maps, list(range(N_CORES)), trace=trace)
    global last_results
    last_results = r
    res = r.results

    # combine: sum the NSH partial-y shards per expert, apply gates
    acc = [None] * N_EXPERTS
    for c in range(N_CORES):
        for j in range(NSLOT):
            e, q = slot_items[j][c]
            cnt = len(tok_lists[e])
            # y[j] is [128, KD, C]: [p, kd, c] = y[kd*128+p, c]
            Yj = np.asarray(res[c][f"y{j}"]).astype(np.float32)
            part = Yj.transpose(1, 0, 2).reshape(D_MODEL, -1)[:, :cnt]
            if acc[e] is None:
                acc[e] = part.copy()
            else:
                acc[e] += part
    out = np.zeros((n_tok, D_MODEL), np.float32)
    for e in range(N_EXPERTS):
        toks = tok_lists[e]
        if len(toks) == 0:
            continue
        contrib = acc[e].T * wt_lists[e][:, None]
        if b2[e].any():
            contrib = contrib + wt_lists[e][:, None] * b2[e][None, :]
        out[toks] += contrib  # token ids unique within one expert
    return out.reshape(B, T, D_MODEL)


# revision 33
# speedup vs baseline: 1.0101x; 1.0101x over previous
"""MoE layer (16 experts, top-2) on 8 Trainium2 NeuronCores, FF-shard parallel.

Strategy:
  - Host computes gating (logits -> top-k -> softmax) and buckets tokens by
    expert (dispatch).
  - Each expert's FFN is split into NSH column shards (D_FF/NSH columns of
    W1 / rows of W2).  All 16*NSH shard-items are sorted by token count and
    dealt into NSLOT = 16*NSH/8 slots of 8 cores each; a slot's compile-time
    width is the max count in its group.  This nearly removes the padding
    imbalance of whole-expert slots (sum of slot widths ~1037 token-pairs
    per core for NSH=4 vs 1078, ideal 1024) at no extra weight traffic:
    shard weights shrink by NSH.
  - DMA is descriptor-rate bound (~25ns + bytes/24GBps per descriptor per
    engine), so every tensor is laid out partition-major with the whole
    slot's data contiguous per partition: x, w1, w2, y all move in 1-2
    DMAs of 8-16KB-per-partition runs (128 descriptors each) per slot.
  - Per slot: ht = silu(W1sh.T @ xt + b1sh) (f on partitions), then partial
    y = W2sh.T @ ht (d on partitions).  Host sums the NSH partials per
    expert and applies the gate weights (combine).
  - All matmuls bf16 (full PE rate, FWL), fp32 PSUM.  y stored bf16
    (partials are small; rounding adds ~0.1% rms, inside the budget).
  - Startup: first W1 quarter rides scalar in parallel with x kd-chunks on
    sync; first mm1 group sub-tiled so the PE starts on the first ~40KB.
    Drain: last slot's final kd group sub-tiled, stores rotate engines.
"""

import math
import os

import numpy as np

D_MODEL = 1024
D_FF = 4096
N_EXPERTS = 16
N_CORES = 8
KD = D_MODEL // 128  # 8 contraction chunks for mm1 / output chunks for mm2

NSH = int(os.environ.get("MOE_NSH", "8"))  # FF shards per expert

_PROG_CACHE: dict[tuple, object] = {}


def _split_tokens(c, cap=512):
    """Split token count c into moving-dim tiles <= cap (PSUM bank limit),
    as equal as possible."""
    n = max(1, math.ceil(c / cap))
    q, r = divmod(c, n)
    sizes = [q + (1 if i < r else 0) for i in range(n)]
    out = []
    c0 = 0
    for sz in sizes:
        out.append((c0, sz))
        c0 += sz
    return out


def _build_program(CS, nsh):
    import concourse.bass as bass  # noqa: F401
    import concourse.tile as tile
    from concourse import bacc, mybir

    f32 = mybir.dt.float32
    bf16 = mybir.dt.bfloat16
    silu = mybir.ActivationFunctionType.Silu

    KFS = (D_FF // nsh) // 128  # f chunks per shard
    NSLOT = len(CS)

    nc = bacc.Bacc("TRN2", target_bir_lowering=False, debug=False, num_devices=N_CORES)

    xt_d, y_d, w1_d, w2_d = [], [], [], []
    for j, C in enumerate(CS):
        xt_d.append(nc.dram_tensor(f"xt{j}", [128, KD, C], bf16, kind="ExternalInput").ap())
        y_d.append(nc.dram_tensor(f"y{j}", [128, KD, C], bf16, kind="ExternalOutput").ap())
        w1_d.append(nc.dram_tensor(f"w1_{j}", [128, KFS, KD, 128], bf16, kind="ExternalInput").ap())
        w2_d.append(nc.dram_tensor(f"w2_{j}", [128, KD, KFS, 128], bf16, kind="ExternalInput").ap())
    b1_d = nc.dram_tensor("b1", [128, NSLOT * KFS], f32, kind="ExternalInput").ap()

    with tile.TileContext(nc) as tc:
        with (
            tc.tile_pool(name="xtp", bufs=3) as xtp,
            tc.tile_pool(name="w1p", bufs=2) as w1p,
            tc.tile_pool(name="w2p", bufs=2) as w2p,
            tc.tile_pool(name="htp", bufs=1) as htp,
            tc.tile_pool(name="ytp", bufs=2) as ytp,
            tc.tile_pool(name="smallp", bufs=1) as smallp,
            tc.tile_pool(name="ps1", bufs=4, space="PSUM") as ps1,
            tc.tile_pool(name="ps2", bufs=4, space="PSUM") as ps2,
        ):
            b1t = smallp.tile([128, NSLOT * KFS], f32, name="b1t", tag="b1t")
            xts = {}

            def load_x(j):
                C = CS[j]
                xts[j] = xtp.tile([128, KD, C], bf16, name=f"xt{j}", tag="xt")
                eng = nc.gpsimd if j % 2 == 0 else nc.scalar
                eng.dma_start(xts[j][:], xt_d[j][:])

            for j, C in enumerate(CS):
                tiles = _split_tokens(C)
                last_slot = j == NSLOT - 1

                # ---- loads ----
                w1t = w1p.tile([128, KFS, KD, 128], bf16, name=f"w1t{j}", tag="w1t")
                w2t = w2p.tile([128, KD, KFS, 128], bf16, name=f"w2t{j}", tag="w2t")
                if j == 0:
                    # startup choreography: mm1 of slot 0 runs one token-half
                    # per pass, so only the left-half x chunks are needed up
                    # front.  Every ring leads with a first-needed item (x
                    # kd7-chunk on sync, x kd5 on scalar, w1 kf0 on gpsimd)
                    # and w1 kf1/kf2 are spread over scalar/gpsimd so the
                    # pass-0 kf cadence (~1us/chunk) can be met while sync
                    # still carries x.  w2 halves trail scalar+gpsimd.
                    xts[0] = xtp.tile([128, KD, C], bf16, name="xt0", tag="xt")
                    xt0 = xts[0]
                    h = (C // 2 + 1) // 2 * 2
                    nc.gpsimd.dma_start(w1t[:, 0:1], w1_d[0][:, 0:1])
                    nc.sync.dma_start(xt0[:, 6:8, 0:h], xt_d[0][:, 6:8, 0:h])
                    nc.scalar.dma_start(xt0[:, 4:6, 0:h], xt_d[0][:, 4:6, 0:h])
                    nc.gpsimd.dma_start(xt0[:, 2:4, 0:h], xt_d[0][:, 2:4, 0:h])
                    nc.scalar.dma_start(xt0[:, 0:2, 0:h], xt_d[0][:, 0:2, 0:h])
                    nc.sync.dma_start(b1t[:], b1_d)
                    if KFS > 1:
                        nc.scalar.dma_start(w1t[:, 1:2], w1_d[0][:, 1:2])
                    if KFS > 2:
                        nc.gpsimd.dma_start(w1t[:, 2:3], w1_d[0][:, 2:3])
                    nc.sync.dma_start(xt0[:, 6:8, h:C], xt_d[0][:, 6:8, h:C])
                    nc.gpsimd.dma_start(xt0[:, 4:6, h:C], xt_d[0][:, 4:6, h:C])
                    nc.gpsimd.dma_start(xt0[:, 2:4, h:C], xt_d[0][:, 2:4, h:C])
                    nc.scalar.dma_start(xt0[:, 0:2, h:C], xt_d[0][:, 0:2, h:C])
                    for kf in range(3, KFS):
                        nc.sync.dma_start(w1t[:, kf : kf + 1], w1_d[0][:, kf : kf + 1])
                    nc.scalar.dma_start(w2t[:, 0:4], w2_d[0][:, 0:4])
                    nc.gpsimd.dma_start(w2t[:, 4:KD], w2_d[0][:, 4:KD])
                else:
                    h = KFS // 2
                    nc.sync.dma_start(w1t[:, 0:h], w1_d[j][:, 0:h])
                    nc.sync.dma_start(w1t[:, h:KFS], w1_d[j][:, h:KFS])
                    # w2 halves on scalar+gpsimd (off sync, which carries w1)
                    nc.scalar.dma_start(w2t[:, 0:4], w2_d[j][:, 0:4])
                    nc.gpsimd.dma_start(w2t[:, 4:KD], w2_d[j][:, 4:KD])

                # ---- mm1: ht[f, c] = silu(W1sh.T @ xt + b1sh) ----
                xt = xts[j]
                ht = htp.tile([128, KFS, C], bf16, name=f"ht{j}", tag="ht")
                # slot 0 runs mm1 one token-tile per pass (so only part of x
                # is needed up front); later slots do all tiles per kf group
                passes = [[t] for t in tiles] if j == 0 else [tiles]
                for pi, ptiles in enumerate(passes):
                    for kf in range(KFS):
                        # sub-tile the very first group so the PE starts as
                        # soon as the first slice of x lands
                        if j == 0 and pi == 0 and kf == 0:
                            c0_, tw_ = ptiles[0]
                            mtiles = [(c0_ + o, w) for o, w in _split_tokens(tw_, 144)]
                        else:
                            mtiles = ptiles
                        pt = [
                            ps1.tile([128, 512], f32, name=f"ps1_{j}_{pi}_{kf}_{i}", tag="ps1")
                            for i in range(len(mtiles))
                        ]
                        for jj, kd in enumerate(reversed(range(KD))):
                            for p, (c0, tw) in zip(pt, mtiles):
                                nc.tensor.matmul(
                                    p[:, :tw],
                                    lhsT=w1t[:, kf, kd],
                                    rhs=xt[:, kd, c0 : c0 + tw],
                                    start=(jj == 0),
                                    stop=(jj == KD - 1),
                                )
                        for p, (c0, tw) in zip(pt, mtiles):
                            nc.scalar.activation(
                                ht[:, kf, c0 : c0 + tw],
                                p[:, :tw],
                                silu,
                                bias=b1t[:, j * KFS + kf : j * KFS + kf + 1],
                            )

                # next slot's x: issued here so it queues behind this slot's
                # w2 halves on its ring and lands during this slot's mm2
                if j + 1 <= NSLOT - 1:
                    load_x(j + 1)

                # ---- mm2: y[d, c] = W2sh.T @ ht (partial over this shard) ----
                yt = ytp.tile([128, KD, C], bf16, name=f"yt{j}", tag="yt")
                for kd in range(KD):
                    fine = last_slot and kd == KD - 1
                    mtiles = _split_tokens(C, 224) if fine else tiles
                    pt2 = [
                        ps2.tile([128, 512], f32, name=f"ps2_{j}_{kd}_{i}", tag="ps2")
                        for i in range(len(mtiles))
                    ]
                    for kf in range(KFS):
                        for p, (c0, tw) in zip(pt2, mtiles):
                            nc.tensor.matmul(
                                p[:, :tw],
                                lhsT=w2t[:, kd, kf],
                                rhs=ht[:, kf, c0 : c0 + tw],
                                start=(kf == 0),
                                stop=(kf == KFS - 1),
                            )
                    for i, (p, (c0, tw)) in enumerate(zip(pt2, mtiles)):
                        if fine:
                            ceng = nc.vector if i % 2 == 0 else nc.scalar
                            if ceng is nc.vector:
                                ceng.tensor_copy(yt[:, kd, c0 : c0 + tw], p[:, :tw])
                            else:
                                ceng.copy(yt[:, kd, c0 : c0 + tw], p[:, :tw])
                            # sync/scalar only: gpsimd's end-of-program ring
                            # drain is slow, keep its last DMA early
                            deng = (nc.sync, nc.scalar, nc.sync, nc.scalar)[i % 4]
                            deng.dma_start(y_d[j][:, kd, c0 : c0 + tw], yt[:, kd, c0 : c0 + tw])
                        else:
                            nc.vector.tensor_copy(yt[:, kd, c0 : c0 + tw], p[:, :tw])
                    # whole-half stores: one DMA per 4 kd chunks (contiguous
                    # per partition), alternating rings per slot.  The last
                    # slot's stores stay OFF gpsimd (its end-of-program ring
                    # drain is ~5us after its last DMA) — sync is free there
                    # since the w1 stream is done.
                    if kd == 3:
                        e1 = nc.sync if last_slot else (nc.scalar if j % 2 == 0 else nc.gpsimd)
                        e1.dma_start(y_d[j][:, 0:4], yt[:, 0:4])
                    elif kd == KD - 1 and not last_slot:
                        e2 = nc.gpsimd if j % 2 == 0 else nc.scalar
                        e2.dma_start(y_d[j][:, 4:KD], yt[:, 4:KD])
                    elif last_slot and kd == KD - 2:
                        nc.scalar.dma_start(y_d[j][:, 4:KD - 1], yt[:, 4:KD - 1])

    nc.compile()
    return nc


def _route(x2d, Wg, k):
    logits = x2d.astype(np.float32) @ Wg.astype(np.float32)  # [T, E]
    idx = np.argsort(-logits, axis=1, kind="stable")[:, :k]  # [T, k]
    vals = np.take_along_axis(logits, idx, axis=1)
    e = np.exp(vals - vals.max(axis=1, keepdims=True))
    w = (e / e.sum(axis=1, keepdims=True)).astype(np.float32)
    return idx, w


def kernel(x, W1, b1, W2, b2, Wg, k):
    import ml_dtypes
    from concourse.bass_utils import run_bass_kernel_spmd

    bf16 = ml_dtypes.bfloat16

    x = np.asarray(x, np.float32)
    W1 = np.asarray(W1, np.float32)
    b1 = np.asarray(b1, np.float32)
    W2 = np.asarray(W2, np.float32)
    b2 = np.asarray(b2, np.float32)
    Wg = np.asarray(Wg, np.float32)
    k = int(k)

    B, T, D = x.shape
    x2d = np.ascontiguousarray(x.reshape(-1, D))
    n_tok = x2d.shape[0]

    idx, w = _route(x2d, Wg, k)

    # bucket tokens per expert
    tok_lists, wt_lists = [], []
    for e in range(N_EXPERTS):
        sel = np.nonzero(idx == e)
        tok_lists.append(sel[0].astype(np.int64))
        wt_lists.append(w[sel[0], sel[1]])

    counts = np.array([len(t) for t in tok_lists])

    # shard items: NSH FF-shards per expert, sorted by count desc, dealt into
    # NSLOT groups of 8 (one item per core per slot)
    items = [(e, q) for e in range(N_EXPERTS) for q in range(NSH)]
    items.sort(key=lambda t: (-counts[t[0]], t[0], t[1]))
    NSLOT = len(items) // N_CORES

    def _pad(c):
        return max(16, ((int(c) + 1) // 2) * 2)

    slot_items, CS = [], []
    for j in range(NSLOT):
        grp = items[N_CORES * j : N_CORES * (j + 1)]
        slot_items.append(grp)
        CS.append(_pad(max(counts[e] for e, _ in grp)))

    key = (NSH, tuple(CS))
    nc = _PROG_CACHE.get(key)
    if nc is None:
        nc = _build_program(CS, NSH)
        _PROG_CACHE[key] = nc

    # host-side weight relayout (bf16, partition-major contiguous blocks)
    KFS = (D_FF // NSH) // 128
    # w1_host[e, q]: [128, KFS, KD, 128];  [p, kf, kd, p2] = W1[e, kd*128+p, q*FSH+kf*128+p2]
    w1_host = np.ascontiguousarray(
        W1.reshape(N_EXPERTS, KD, 128, NSH, KFS, 128).transpose(0, 3, 2, 4, 1, 5)
    ).astype(bf16)
    # w2_host[e, q]: [128, KD, KFS, 128];  [p, kd, kf, p2] = W2[e, q*FSH+kf*128+p, kd*128+p2]
    w2_host = np.ascontiguousarray(
        W2.reshape(N_EXPERTS, NSH, KFS, 128, KD, 128).transpose(0, 1, 3, 4, 2, 5)
    ).astype(bf16)
    # b1_host[e, q]: [128, KFS]
    b1_host = np.ascontiguousarray(
        b1.reshape(N_EXPERTS, NSH, KFS, 128).transpose(0, 1, 3, 2)
    )
    x_bf = x2d.astype(bf16)

    in_maps = []
    for c in range(N_CORES):
        b1all = np.zeros((128, NSLOT * KFS), np.float32)
        m = {"b1": b1all}
        for j in range(NSLOT):
            e, q = slot_items[j][c]
            toks = tok_lists[e]
            cnt = len(toks)
            C = CS[j]
            xt = np.zeros((128, KD, C), bf16)
            # xt[p, kd, c] = x[token c, kd*128 + p]
            xt[:, :, :cnt] = x_bf[toks].reshape(cnt, KD, 128).transpose(2, 1, 0)
            m[f"xt{j}"] = xt
            m[f"w1_{j}"] = w1_host[e, q]
            m[f"w2_{j}"] = w2_host[e, q]
            b1all[:, j * KFS : (j + 1) * KFS] = b1_host[e, q]
        in_maps.append(m)

    trace = bool(os.environ.get("MOE_TRACE"))
    r = run_bass_kernel_spmd(nc, in_maps, list(range(N_CORES)), trace=trace)
    global last_results
    last_results = r
    res = r.results

    # combine: sum the NSH partial-y shards per expert, apply gates
    acc = [None] * N_EXPERTS
    for c in range(N_CORES):
        for j in range(NSLOT):
            e, q = slot_items[j][c]
            cnt = len(tok_lists[e])
            # y[j] is [128, KD, C]: [p, kd, c] = y[kd*128+p, c]
            Yj = np.asarray(res[c][f"y{j}"]).astype(np.float32)
            part = Yj.transpose(1, 0, 2).reshape(D_MODEL, -1)[:, :cnt]
            if acc[e] is None:
                acc[e] = part.copy()
            else:
                acc[e] += part
    out = np.zeros((n_tok, D_MODEL), np.float32)
    for e in range(N_EXPERTS):
        toks = tok_lists[e]
        if len(toks) == 0:
            continue
        contrib = acc[e].T * wt_lists[e][:, None]
        if b2[e].any():
            contrib = contrib + wt_lists[e][:, None] * b2[e][None, :]
        out[toks] += contrib  # token ids unique within one expert
    return out.reshape(B, T, D_MODEL)
